# revision 6
# baseline (speedup 1.0000x reference)
"""Multi-head attention (B=2, S=4096, D=512, H=8) on 8 TRN2 NeuronCores.

Sharding: each core owns (batch b = core//4, head-pair hp = core%4).
 - Q/K/V are batch-sharded (cores sharing b get the same activations,
   pre-transposed to [D, S] so the PE can contract over D).
 - W_q/W_k/W_v column-sharded per head-pair (128 output dims / core).
 - W_o row-sharded: each core emits a partial [S, D] output which the
   host sums per batch (the all-reduce of the row-sharded matmul).

Device pipeline per core (all matmuls bf16, fp32 accumulation):
  1. qT/kT = (W_x.T slice).T @ X.T -> [128(d), S] PSUM, +bias, cast bf16.
     v = X.T-tiles.T @ W_v.T slice -> [S(part), 128(d)], +bias via a
     rank-1 ones matmul, then multiplied by the key mask and stored as
     [k, 1+64] tiles whose column 0 holds the mask value itself (this
     single column yields the softmax denominator AND applies the mask).
  2. Per head / 512-wide q-chunk: S_T[k,q] = kT-tile.T @ qT (scores,
     transposed, so no transposes are needed anywhere), exp via ScalarE
     on 3-bank PSUM chunks (scale=1/8 folded in), P -> bf16 SBUF,
     acc[1+64, q] += v_aug.T @ P accumulated over all k-tiles.
     Row 0 of acc = sum_k mask*exp = softmax denominator. attn_T =
     acc[1:65]/acc[0] (reciprocal + gpsimd partition-broadcast + DVE),
     row 0 of the stored attn tile is set to 1.0.
  3. out[s,e] = sum_h [1|attn_T_h].T @ [b_o-or-0|W_o slice.T] -> partial
     output rows, DMA to DRAM.
"""

import sys

for _p in ("/opt/trn_rl_repo", "/opt/pypackages"):
    if _p not in sys.path:
        sys.path.append(_p)

import dataclasses

import numpy as np
import ml_dtypes

B = 2
S = 4096
D = 512
H = 8
DK = 64
N_CORES = 8

P = 128          # partitions
QC = 512         # q-chunk width
N_SC = S // QC   # 8 q-chunks
N_ST = S // P    # 32 s-tiles
N_DT = D // P    # 4 D-tiles (contraction tiles for projections)

_COMPILED = {}
_LAST_IN_MAPS = None
_LAST_RESULTS = None


def _build(nkt: int):
    """Build + compile the per-core bass program. nkt = number of
    128-wide key tiles (32 = full, smaller when keys are compacted)."""
    import concourse.bass as bass  # noqa: F401
    import concourse.mybir as mybir
    import concourse.tile as tile
    from concourse import bacc

    f32 = mybir.dt.float32
    bf16 = mybir.dt.bfloat16
    EXP = mybir.ActivationFunctionType.Exp

    nk = nkt * P

    nc = bacc.Bacc("TRN2", target_bir_lowering=False, debug=False,
                   enable_asserts=False)

    QT = nc.dram_tensor("QT", [D, S], bf16, kind="ExternalInput").ap()
    KT = nc.dram_tensor("KT", [D, nk], bf16, kind="ExternalInput").ap()
    VT = nc.dram_tensor("VT", [D, nk], bf16, kind="ExternalInput").ap()
    WQT = nc.dram_tensor("WQT", [D, P], bf16, kind="ExternalInput").ap()
    WKT = nc.dram_tensor("WKT", [D, P], bf16, kind="ExternalInput").ap()
    WVT = nc.dram_tensor("WVT", [D, P], bf16, kind="ExternalInput").ap()
    WOB = nc.dram_tensor("WOB", [2, 65, D], bf16, kind="ExternalInput").ap()
    BQ = nc.dram_tensor("BQ", [P, 1], f32, kind="ExternalInput").ap()
    BK = nc.dram_tensor("BK", [P, 1], f32, kind="ExternalInput").ap()
    BV = nc.dram_tensor("BV", [1, P], f32, kind="ExternalInput").ap()
    MASKF = nc.dram_tensor("MASKF", [P, nkt], f32, kind="ExternalInput").ap()
    OUT = nc.dram_tensor("OUT", [S, D], f32, kind="ExternalOutput").ap()

    with tile.TileContext(nc) as tc:
        with tc.tile_pool(name="persist", bufs=1) as persist:
            # ---- persistent SBUF tensors ----
            qT_sb = persist.tile([P, S], bf16)      # [d(2 heads), s]
            kT_sb = persist.tile([P, nk], bf16)     # [d(2 heads), k]
            v0_sb = persist.tile([P, nkt, 65], bf16)  # head0 [k, kt, mask|v]
            v1_sb = persist.tile([P, nkt, 65], bf16)  # head1
            a0_sb = persist.tile([65, S], bf16)     # head0 [1|attn_T, q]
            a1_sb = persist.tile([65, S], bf16)     # head1
            wq_sb = persist.tile([P, N_DT, P], bf16)
            wk_sb = persist.tile([P, N_DT, P], bf16)
            wv_sb = persist.tile([P, N_DT, P], bf16)
            wo0_sb = persist.tile([65, D], bf16)
            wo1_sb = persist.tile([65, D], bf16)
            bq_sb = persist.tile([P, 1], f32)
            bk_sb = persist.tile([P, 1], f32)
            bv_sb = persist.tile([1, P], f32)
            ones_sb = persist.tile([1, P], f32)
            mask_sb = persist.tile([P, nkt], f32)

            nc.vector.memset(ones_sb, 1.0)
            for dt in range(N_DT):
                dsl = slice(dt * P, (dt + 1) * P)
                nc.sync.dma_start(out=wq_sb[:, dt, :], in_=WQT[dsl, :])
                nc.sync.dma_start(out=wk_sb[:, dt, :], in_=WKT[dsl, :])
                nc.sync.dma_start(out=wv_sb[:, dt, :], in_=WVT[dsl, :])
            nc.sync.dma_start(out=wo0_sb, in_=WOB[0])
            nc.sync.dma_start(out=wo1_sb, in_=WOB[1])
            nc.sync.dma_start(out=bq_sb, in_=BQ)
            nc.sync.dma_start(out=bk_sb, in_=BK)
            nc.sync.dma_start(out=bv_sb, in_=BV)
            nc.sync.dma_start(out=mask_sb, in_=MASKF)
            # column 64 of every v tile = mask value (denominator source)
            nc.vector.tensor_copy(out=v0_sb[:, :, 64], in_=mask_sb)
            nc.vector.tensor_copy(out=v1_sb[:, :, 64], in_=mask_sb)

            # ================= phase P: projections =================
            with tc.tile_pool(name="pin", bufs=4) as pin, \
                 tc.tile_pool(name="pps", bufs=2, space="PSUM") as pps:
                # q and k: psum[d2h, s-chunk] = sum_dt WxT_dt.T @ XT_dt
                for x_dram, w_sb, b_sb, x_out in (
                    (KT, wk_sb, bk_sb, kT_sb),
                    (QT, wq_sb, bq_sb, qT_sb),
                ):
                    n_sc = x_dram.shape[1] // QC
                    for sc in range(n_sc):
                        ssl = slice(sc * QC, (sc + 1) * QC)
                        ps = pps.tile([P, QC], f32, tag="qk")
                        for dt in range(N_DT):
                            xin = pin.tile([P, QC], bf16, tag="qkin")
                            nc.sync.dma_start(
                                out=xin, in_=x_dram[dt * P:(dt + 1) * P, ssl])
                            nc.tensor.matmul(ps, lhsT=w_sb[:, dt, :], rhs=xin,
                                             start=(dt == 0), stop=(dt == N_DT - 1))
                        nc.vector.tensor_scalar_add(
                            out=x_out[:, ssl], in0=ps, scalar1=b_sb)
                # v: psum[k-tile, d2h] = sum_dt VT_dt_tile.T @ WvT_dt (+ bias)
                for st in range(nkt):
                    ps = pps.tile([P, P], f32, tag="v")
                    for dt in range(N_DT):
                        vin = pin.tile([P, P], bf16, tag="vin")
                        nc.sync.dma_start(
                            out=vin,
                            in_=VT[dt * P:(dt + 1) * P, st * P:(st + 1) * P])
                        nc.tensor.matmul(ps, lhsT=vin, rhs=wv_sb[:, dt, :],
                                         start=(dt == 0), stop=False)
                    nc.tensor.matmul(ps, lhsT=ones_sb, rhs=bv_sb,
                                     start=False, stop=True)
                    msl = mask_sb[:, st:st + 1]
                    nc.vector.tensor_scalar_mul(
                        out=v0_sb[:, st, 0:DK], in0=ps[:, 0:DK], scalar1=msl)
                    nc.vector.tensor_scalar_mul(
                        out=v1_sb[:, st, 0:DK], in0=ps[:, DK:P], scalar1=msl)

            # ================= phase A: attention =================
            # k-tile rounds of <=3 (exp reads 3 PSUM banks at once)
            rounds = []
            ki = 0
            while ki < nkt:
                n = min(3, nkt - ki)
                rounds.append((ki, n))
                ki += n

            with tc.tile_pool(name="aps", bufs=1, space="PSUM") as aps, \
                 tc.tile_pool(name="adr", bufs=2, space="DRAM") as adr, \
                 tc.tile_pool(name="asb", bufs=1) as asb:
                for h, (v_sb, a_sb) in enumerate(((v0_sb, a0_sb), (v1_sb, a1_sb))):
                    hq = slice(h * DK, (h + 1) * DK)
                    for sc in range(N_SC):
                        ssl = slice(sc * QC, (sc + 1) * QC)
                        acc = aps.tile([P, QC], f32, tag="acc", bufs=1)
                        for (k0, nk_r) in rounds:
                            s_ps = aps.tile([P, 3 * QC], f32, tag="mm1", bufs=2)
                            for j in range(nk_r):
                                kt = k0 + j
                                nc.tensor.matmul(
                                    s_ps[:, j * QC:(j + 1) * QC],
                                    lhsT=kT_sb[hq, kt * P:(kt + 1) * P],
                                    rhs=qT_sb[hq, ssl],
                                    start=True, stop=True)
                            p_sb = asb.tile([P, 3 * QC], bf16, tag="p", bufs=3)
                            nc.scalar.activation(
                                p_sb[:, :nk_r * QC], s_ps[:, :nk_r * QC],
                                EXP, bias=0.0, scale=0.125)
                            for j in range(nk_r):
                                kt = k0 + j
                                nc.tensor.matmul(
                                    acc[0:65, :],
                                    lhsT=v_sb[:, kt, :],
                                    rhs=p_sb[:, j * QC:(j + 1) * QC],
                                    start=(kt == 0), stop=(kt == nkt - 1))
                        # normalize: attn = acc[0:64] / acc[64]
                        rec = asb.tile([65, QC], f32, tag="rec", bufs=2)
                        nc.vector.reciprocal(out=rec[64:65, :], in_=acc[64:65, :])
                        # broadcast 1/denom across partitions via a DRAM
                        # bounce (gpsimd partition_broadcast misreads
                        # nonzero base partitions on HW; SBUF DMA sources
                        # reject zero partition stride, DRAM sources don't)
                        dsc = adr.tile([1, QC], f32, tag="dsc", bufs=2)
                        nc.sync.dma_start(out=dsc, in_=rec[64:65, :])
                        recb = asb.tile([64, QC], f32, tag="recb", bufs=2)
                        dsc_b = dataclasses.replace(
                            dsc, ap=[[0, 64], list(dsc.ap[-1])])
                        nc.sync.dma_start(out=recb, in_=dsc_b)
                        nc.vector.tensor_mul(
                            out=a_sb[0:64, ssl], in0=acc[0:64, :],
                            in1=recb)
                        nc.vector.memset(a_sb[64:65, ssl], 1.0)

                        # ---- output projection (after both heads did sc) ----
                        if h == 1:
                            for st4 in range(QC // P):
                                st = sc * (QC // P) + st4
                                tsl = slice(st * P, (st + 1) * P)
                                po = aps.tile([P, D], f32, tag="oproj", bufs=1)
                                nc.tensor.matmul(po, lhsT=a0_sb[:, tsl],
                                                 rhs=wo0_sb, start=True,
                                                 stop=False)
                                nc.tensor.matmul(po, lhsT=a1_sb[:, tsl],
                                                 rhs=wo1_sb, start=False,
                                                 stop=True)
                                osb = asb.tile([P, D], f32, tag="osb", bufs=3)
                                nc.vector.tensor_copy(out=osb, in_=po)
                                nc.sync.dma_start(out=OUT[tsl, :], in_=osb)

    nc.compile()
    return nc


def _get_compiled(nkt: int):
    if nkt not in _COMPILED:
        _COMPILED[nkt] = _build(nkt)
    return _COMPILED[nkt]


def kernel(Q, K, V, mask, W_q, b_q, W_k, b_k, W_v, b_v, W_o, b_o):
    from concourse import bass_utils

    bf16 = ml_dtypes.bfloat16
    nkt = N_ST
    nk = nkt * P

    Q = np.asarray(Q, np.float32)
    K = np.asarray(K, np.float32)
    V = np.asarray(V, np.float32)
    mask = np.asarray(mask)
    W_q = np.asarray(W_q, np.float32)
    W_k = np.asarray(W_k, np.float32)
    W_v = np.asarray(W_v, np.float32)
    W_o = np.asarray(W_o, np.float32)
    b_q = np.asarray(b_q, np.float32)
    b_k = np.asarray(b_k, np.float32)
    b_v = np.asarray(b_v, np.float32)
    b_o = np.asarray(b_o, np.float32)

    nc = _get_compiled(nkt)

    in_maps = []
    for core in range(N_CORES):
        b = core // 4
        hp = core % 4
        rsl = slice(hp * P, (hp + 1) * P)   # rows of W_q/W_k/W_v, 2 heads
        h0, h1 = 2 * hp, 2 * hp + 1

        qt = np.ascontiguousarray(Q[b].T).astype(bf16)
        kt = np.ascontiguousarray(K[b].T).astype(bf16)
        vt = np.ascontiguousarray(V[b].T).astype(bf16)

        wob = np.zeros((2, 65, D), np.float32)
        if hp == 0:
            wob[0, 64, :] = b_o
        wob[0, 0:64, :] = W_o[:, h0 * DK:(h0 + 1) * DK].T
        wob[1, 0:64, :] = W_o[:, h1 * DK:(h1 + 1) * DK].T

        maskf = np.ascontiguousarray(
            mask[b].astype(np.float32).reshape(nkt, P).T)

        in_maps.append({
            "QT": qt,
            "KT": kt,
            "VT": vt,
            "WQT": np.ascontiguousarray(W_q[rsl, :].T).astype(bf16),
            "WKT": np.ascontiguousarray(W_k[rsl, :].T).astype(bf16),
            "WVT": np.ascontiguousarray(W_v[rsl, :].T).astype(bf16),
            "WOB": wob.astype(bf16),
            "BQ": np.ascontiguousarray(b_q[rsl].reshape(P, 1)),
            "BK": np.ascontiguousarray(b_k[rsl].reshape(P, 1)),
            "BV": np.ascontiguousarray(b_v[rsl].reshape(1, P)),
            "MASKF": maskf,
        })

    global _LAST_IN_MAPS, _LAST_RESULTS
    _LAST_IN_MAPS = in_maps

    res = bass_utils.run_bass_kernel_spmd(
        nc, in_maps, core_ids=list(range(N_CORES)))

    _LAST_RESULTS = res.results

    out = np.zeros((B, S, D), np.float32)
    for core in range(N_CORES):
        out[core // 4] += np.asarray(res.results[core]["OUT"], np.float32)
    return out
